# revision 13
# baseline (speedup 1.0000x reference)
"""Trainium2 Bass kernel for CausalAnalysisHierPredictor (scene-graph hier head).

Strategy
--------
Data-parallel over the pair dimension E=32768 across 8 NeuronCores
(4096 pairs/core). All gathers, transposes, and algebraic weight folding
happen on the host; the device runs only dense GEMMs + ReLU + adds.

Algebra (exact restructuring of the reference):
  post_ctx  = relu(ctx_rep @ W_post_cat + b_post_cat)
            = relu(ec[hidx] @ W_h + ec[tidx] @ W_t + b_comb)
      with W_h = Wpe[:, :512] @ Wpc[:512],  W_t = Wpe[:, 512:] @ Wpc[512:]
           b_comb = bpe[:512] @ Wpc[:512] + bpe[512:] @ Wpc[512:] + bpc
  out54     = post_ctx @ [Wc1|Wc2|Wc3|Wcs] + vis @ [Wv1|Wv2|Wv3|Wvs]
              + table54[sub*151 + obj]
      where table54 folds the GEO/POS/SEM column permutation, the
      log-sum-exp "sup" columns and all head biases into one 22801x54
      lookup table (pure per-row function of freq_table).

Device layout per core (pairs on the matmul free axis):
  post_ctx[pool, pair] accumulated in PSUM from stationary W chunks
  x moving X^T chunks [feat, pair512]; ReLU+bias via ScalarE into bf16
  SBUF; both heads accumulate into one PSUM tile [54, 512] per block
  with stationary Wc/Wv chunks [pool128, 54].

  Main GEMM runs in fp8e4m3 with DoubleRow (K=256/matmul); inputs are
  pre-scaled on host (X*16, W*64) to sit in e4m3's normal range, and the
  1024x PSUM scale is folded into the ReLU bias (relu is positively
  homogeneous) and into Wc (divided by 1024). Heads stay bf16.
  Pair blocks are processed two at a time so each LDWEIGHTS feeds two
  matmuls; each block's vis-head matmuls run before the mains so the
  first block can start before the big W tiles finish loading.
"""

import sys

if "/opt/trn_rl_repo" not in sys.path:
    sys.path.insert(0, "/opt/trn_rl_repo")

import numpy as np
import ml_dtypes

BF16 = ml_dtypes.bfloat16
FP8 = ml_dtypes.float8_e4m3
F32 = np.float32

USE_FP8 = True             # fp8e4m3 DoubleRow main GEMM (heads stay bf16)
XSCALE = 16.0              # host pre-scale for X in fp8 mode
WSCALE = 64.0              # host pre-scale for W_h/W_t in fp8 mode

NCORES = 8
E = 32768
EPC = E // NCORES          # 4096 pairs per core
HID = 512
POOL = 4096
NOBJ = 151
NH = 54                    # 15 + 11 + 24 + 4 head outputs
NB = 8                     # pair blocks per core
BLK = EPC // NB            # 512 pairs per block
KB = HID // 128            # 4 feature chunks of 128
MB = POOL // 128           # 32 pool chunks

GEO = np.array([1, 2, 3, 4, 5, 6, 8, 10, 22, 23, 29, 31, 32, 33, 43], np.int64)
POS = np.array([9, 16, 17, 20, 27, 30, 36, 42, 48, 49, 50], np.int64)
SEM = np.array([7, 11, 12, 13, 14, 15, 18, 19, 21, 24, 25, 26, 28, 34, 35,
                37, 38, 39, 40, 41, 44, 45, 46, 47], np.int64)

_state: dict = {}


def _build():
    """Build + compile the per-core Bass program (same program on all cores)."""
    import concourse.bacc as bacc
    import concourse.mybir as mybir
    from concourse import bass, tile

    dt = mybir.dt
    xdt = dt.float8e4 if USE_FP8 else dt.bfloat16
    nc = bacc.Bacc("TRN2", target_bir_lowering=False, debug=False)

    if USE_FP8:
        # (side, rpair, part, 2, pool)
        w2_d = nc.dram_tensor("w2", [2, KB // 2, 128, 2, POOL], xdt, kind="ExternalInput").ap()
    else:
        w2_d = nc.dram_tensor("w2", [2, KB, 128, POOL], xdt, kind="ExternalInput").ap()
    wc_d = nc.dram_tensor("wc", [128, MB, NH], dt.bfloat16, kind="ExternalInput").ap()
    wv_d = nc.dram_tensor("wv", [128, MB, NH], dt.bfloat16, kind="ExternalInput").ap()
    bc_d = nc.dram_tensor("bcomb", [128, MB], dt.float32, kind="ExternalInput").ap()
    xh_d = nc.dram_tensor("xh", [NB, 128, KB, BLK], xdt, kind="ExternalInput").ap()
    xt_d = nc.dram_tensor("xt", [NB, 128, KB, BLK], xdt, kind="ExternalInput").ap()
    vis_d = nc.dram_tensor("vis", [NB, 128, MB, BLK], dt.bfloat16, kind="ExternalInput").ap()
    b54_d = nc.dram_tensor("b54", [NB, NH, BLK], dt.float32, kind="ExternalInput").ap()
    out_d = nc.dram_tensor("out", [NB, NH, BLK], dt.float32, kind="ExternalOutput").ap()

    NCHUNK = KB // 2 if USE_FP8 else KB   # stationary chunks per side
    DR = mybir.MatmulPerfMode.DoubleRow if USE_FP8 else None

    pdt = dt.bfloat16
    VSPLIT = 8                 # vis DMA split so vis-head MMs start early

    with tile.TileContext(nc) as tc:
        with (
            tc.tile_pool(name="const", bufs=1) as cpool,
            tc.tile_pool(name="xin", bufs=2) as xpool,
            tc.tile_pool(name="vin", bufs=2) as vpool,
            tc.tile_pool(name="bin", bufs=2) as bpool,
            tc.tile_pool(name="post", bufs=2 * MB + 4) as ppool,
            tc.tile_pool(name="outp", bufs=2) as opool,
            tc.tile_pool(name="mm", bufs=4, space=bass.MemorySpace.PSUM) as mmpool,
            tc.tile_pool(name="hp", bufs=3, space=bass.MemorySpace.PSUM) as hppool,
        ):
            wv_t = cpool.tile([128, MB, NH], dt.bfloat16, tag="wv")
            nc.sync.dma_start(wv_t[:], wv_d[:])

            def mk_block_tiles(n):
                vis_t = vpool.tile([128, MB, BLK], dt.bfloat16, tag="vis")
                step = MB // VSPLIT
                for v in range(VSPLIT):
                    nc.sync.dma_start(vis_t[:, v * step:(v + 1) * step, :],
                                      vis_d[n, :, v * step:(v + 1) * step, :])
                xh_t = xpool.tile([128, KB, BLK], xdt, tag="xh")
                nc.sync.dma_start(xh_t[:], xh_d[n])
                xt_t = xpool.tile([128, KB, BLK], xdt, tag="xt")
                nc.sync.dma_start(xt_t[:], xt_d[n])
                b54_t = bpool.tile([NH, BLK], dt.float32, tag="b54")
                nc.sync.dma_start(b54_t[:], b54_d[n])
                hp = hppool.tile([NH, BLK], dt.float32, tag="hp")
                return (n, xh_t, xt_t, vis_t, b54_t, hp)

            # first pair's inputs go into the DMA queues before the big W
            # tiles so the vis heads can start right away
            pending = [mk_block_tiles(0), mk_block_tiles(1)]

            wc_t = cpool.tile([128, MB, NH], dt.bfloat16, tag="wc")
            nc.sync.dma_start(wc_t[:], wc_d[:])
            bc_t = cpool.tile([128, MB], dt.float32, tag="bc")
            nc.sync.dma_start(bc_t[:], bc_d[:])
            w_tiles = []
            for s in range(2):
                row = []
                for k in range(NCHUNK):
                    if USE_FP8:
                        t = cpool.tile([128, 2, POOL], xdt, tag=f"w{s}{k}")
                    else:
                        t = cpool.tile([128, POOL], xdt, tag=f"w{s}{k}")
                    nc.sync.dma_start(t[:], w2_d[s, k])
                    row.append(t)
                w_tiles.append(row)

            for g in range(NB // 2):
                blocks = pending
                # interleave the two blocks' vis-head matmuls so each
                # LDWEIGHTS of a wv chunk feeds two matmuls
                for m in range(MB):
                    for b in range(2):
                        hp, vis_t = blocks[b][5], blocks[b][3]
                        nc.tensor.matmul(hp[:], wv_t[:, m, :], vis_t[:, m, :],
                                         start=(m == 0), stop=False)

                posts = [[None] * MB, [None] * MB]
                for m in range(MB):
                    mps = []
                    for b in range(2):
                        xh_t, xt_t = blocks[b][1], blocks[b][2]
                        mp = mmpool.tile([128, BLK], dt.float32, tag="mp")
                        ctr = 0
                        for s, x_t in ((0, xh_t), (1, xt_t)):
                            for k in range(NCHUNK):
                                if USE_FP8:
                                    lhsT = w_tiles[s][k][:, :, m * 128:(m + 1) * 128]
                                    rhs = x_t[:, 2 * k:2 * k + 2, :]
                                else:
                                    lhsT = w_tiles[s][k][:, m * 128:(m + 1) * 128]
                                    rhs = x_t[:, k, :]
                                nc.tensor.matmul(
                                    mp[:], lhsT, rhs,
                                    start=(ctr == 0), stop=(ctr == 2 * NCHUNK - 1),
                                    perf_mode=DR,
                                )
                                ctr += 1
                        mps.append(mp)
                    for b in range(2):
                        post = ppool.tile([128, BLK], pdt, tag="post")
                        nc.scalar.activation(
                            post[:], mps[b][:], mybir.ActivationFunctionType.Relu,
                            bias=bc_t[:, m:m + 1],
                        )
                        posts[b][m] = post
                # queue next pair's DMAs before the batched ctx matmuls
                if g + 1 < NB // 2:
                    pending = [mk_block_tiles(2 * g + 2), mk_block_tiles(2 * g + 3)]
                # ctx head batched as one contiguous paired run (each wc
                # LDWEIGHTS feeds two matmuls, no ScalarE sync in between)
                for m in range(MB):
                    for b in range(2):
                        hp = blocks[b][5]
                        nc.tensor.matmul(hp[:], wc_t[:, m, :], posts[b][m][:],
                                         start=False, stop=(m == MB - 1))
                for b in range(2):
                    n, _, _, _, b54_t, hp = blocks[b]
                    ot = opool.tile([NH, BLK], dt.float32, tag="ot")
                    nc.vector.tensor_add(ot[:], hp[:], b54_t[:])
                    nc.sync.dma_start(out_d[n], ot[:])

    nc.compile()
    return nc


def _prep(inputs):
    """Host-side: fold weights, build the 54-col bias table, gather, block/transpose."""
    ec = np.ascontiguousarray(np.asarray(inputs["edge_ctx"], F32))
    pidx = np.asarray(inputs["pair_idx"])
    vis = np.asarray(inputs["vis_rep"], F32)
    ppred = np.asarray(inputs["pair_pred"])
    f64 = np.float64
    Wpe = np.asarray(inputs["W_post_emb"], f64)
    bpe = np.asarray(inputs["b_post_emb"], f64)
    Wpc = np.asarray(inputs["W_post_cat"], f64)
    bpc = np.asarray(inputs["b_post_cat"], f64)

    W_h = (Wpe[:, :HID] @ Wpc[:HID]).astype(F32)
    W_t = (Wpe[:, HID:] @ Wpc[HID:]).astype(F32)
    b_comb = (bpe[:HID] @ Wpc[:HID] + bpe[HID:] @ Wpc[HID:] + bpc).astype(F32)

    Wc_all = np.concatenate([np.asarray(inputs["Wc1"], F32), np.asarray(inputs["Wc2"], F32),
                             np.asarray(inputs["Wc3"], F32), np.asarray(inputs["Wcs"], F32)], 1)
    Wv_all = np.concatenate([np.asarray(inputs["Wv1"], F32), np.asarray(inputs["Wv2"], F32),
                             np.asarray(inputs["Wv3"], F32), np.asarray(inputs["Wvs"], F32)], 1)
    bh_all = np.concatenate([
        np.asarray(inputs["bc1"], F32) + np.asarray(inputs["bv1"], F32),
        np.asarray(inputs["bc2"], F32) + np.asarray(inputs["bv2"], F32),
        np.asarray(inputs["bc3"], F32) + np.asarray(inputs["bv3"], F32),
        np.asarray(inputs["bcs"], F32) + np.asarray(inputs["bvs"], F32)])

    T = np.asarray(inputs["freq_table"], f64)
    t1, t2, t3 = T[:, GEO], T[:, POS], T[:, SEM]
    lse = lambda t: np.log(np.exp(t).sum(-1))
    tsup = np.stack([T[:, 0], lse(t1), lse(t2), lse(t3)], 1)
    table54 = np.concatenate([t1, t2, t3, tsup], 1).astype(F32) + bh_all

    lin = ppred[:, 0].astype(np.int64) * NOBJ + ppred[:, 1]
    bias54 = table54[lin]                                   # [E, 54]

    if USE_FP8:
        # fp8 DoubleRow layout: w2[s, r, p, i, col] = W_s[(2r+i)*128+p, col] * WSCALE
        w2 = np.stack([
            (W_h * WSCALE).reshape(KB // 2, 2, 128, POOL).transpose(0, 2, 1, 3),
            (W_t * WSCALE).reshape(KB // 2, 2, 128, POOL).transpose(0, 2, 1, 3),
        ]).astype(FP8)
        psum_scale = XSCALE * WSCALE
        wc = (Wc_all / psum_scale).reshape(MB, 128, NH).transpose(1, 0, 2).astype(BF16)
        bcomb = np.ascontiguousarray((b_comb * psum_scale).reshape(MB, 128).T)
        xdt = FP8
        xs = XSCALE
    else:
        w2 = np.stack([W_h.reshape(KB, 128, POOL),
                       W_t.reshape(KB, 128, POOL)]).astype(BF16)
        wc = Wc_all.reshape(MB, 128, NH).transpose(1, 0, 2).astype(BF16)
        bcomb = np.ascontiguousarray(b_comb.reshape(MB, 128).T)
        xdt = BF16
        xs = 1.0
    wv = Wv_all.reshape(MB, 128, NH).transpose(1, 0, 2).astype(BF16)

    Xh = ec[pidx[:, 0]] * xs                                # [E, 512]
    Xt = ec[pidx[:, 1]] * xs
    xh_dev = Xh.reshape(NCORES, NB, BLK, KB, 128).transpose(0, 1, 4, 3, 2).astype(xdt)
    xt_dev = Xt.reshape(NCORES, NB, BLK, KB, 128).transpose(0, 1, 4, 3, 2).astype(xdt)
    vis_dev = vis.reshape(NCORES, NB, BLK, MB, 128).transpose(0, 1, 4, 3, 2).astype(BF16)
    b54_dev = np.ascontiguousarray(
        bias54.reshape(NCORES, NB, BLK, NH).transpose(0, 1, 3, 2))

    in_maps = []
    for c in range(NCORES):
        in_maps.append({
            "w2": w2, "wc": wc, "wv": wv, "bcomb": bcomb,
            "xh": xh_dev[c], "xt": xt_dev[c], "vis": vis_dev[c],
            "b54": b54_dev[c],
        })
    return in_maps


def kernel(**inputs):
    from concourse.bass_utils import run_bass_kernel_spmd

    nc = _state.get("nc")
    if nc is None:
        nc = _build()
        _state["nc"] = nc

    in_maps = _prep(inputs)
    _state["in_maps"] = in_maps

    res = run_bass_kernel_spmd(nc, in_maps, list(range(NCORES)))
    _state["last_results"] = res

    full = np.concatenate(
        [res.results[c]["out"].transpose(0, 2, 1).reshape(EPC, NH)
         for c in range(NCORES)], 0)
    rel1 = np.ascontiguousarray(full[:, 0:15])
    rel2 = np.ascontiguousarray(full[:, 15:26])
    rel3 = np.ascontiguousarray(full[:, 26:50])
    sup = np.ascontiguousarray(full[:, 50:54])
    return (rel1, rel2, rel3, sup)


# revision 15
# speedup vs baseline: 1.2003x; 1.2003x over previous
"""Trainium2 Bass kernel for CausalAnalysisHierPredictor (scene-graph hier head).

Strategy
--------
Data-parallel over the pair dimension E=32768 across 8 NeuronCores
(4096 pairs/core). All gathers, transposes, and algebraic weight folding
happen on the host; the device runs only dense GEMMs + ReLU + adds.

Algebra (exact restructuring of the reference):
  post_ctx  = relu(ctx_rep @ W_post_cat + b_post_cat)
            = relu(ec[hidx] @ W_h + ec[tidx] @ W_t + b_comb)
      with W_h = Wpe[:, :512] @ Wpc[:512],  W_t = Wpe[:, 512:] @ Wpc[512:]
           b_comb = bpe[:512] @ Wpc[:512] + bpe[512:] @ Wpc[512:] + bpc
  out54     = post_ctx @ [Wc1|Wc2|Wc3|Wcs] + vis @ [Wv1|Wv2|Wv3|Wvs]
              + table54[sub*151 + obj]
      where table54 folds the GEO/POS/SEM column permutation, the
      log-sum-exp "sup" columns and all head biases into one 22801x54
      lookup table (pure per-row function of freq_table).

Device layout per core (pairs on the matmul free axis):
  post_ctx[pool, pair] accumulated in PSUM from stationary W chunks
  x moving X^T chunks [feat, pair512]; ReLU+bias via ScalarE into bf16
  SBUF; both heads accumulate into one PSUM tile [54, 512] per block
  with stationary Wc/Wv chunks [pool128, 54].

  Main GEMM runs in fp8e4m3 with DoubleRow (K=256/matmul); inputs are
  pre-scaled on host (X*16, W*64) to sit in e4m3's normal range, and the
  1024x PSUM scale is folded into the ReLU bias (relu is positively
  homogeneous) and into Wc (divided by 1024). Heads stay bf16.
  Pair blocks are processed two at a time so each LDWEIGHTS feeds two
  matmuls; each block's vis-head matmuls run before the mains so the
  first block can start before the big W tiles finish loading.
"""

import sys

if "/opt/trn_rl_repo" not in sys.path:
    sys.path.insert(0, "/opt/trn_rl_repo")

import numpy as np
import ml_dtypes

BF16 = ml_dtypes.bfloat16
FP8 = ml_dtypes.float8_e4m3
F32 = np.float32

USE_FP8 = True             # fp8e4m3 DoubleRow main GEMM (heads stay bf16)
XSCALE = 16.0              # host pre-scale for X in fp8 mode
WSCALE = 64.0              # host pre-scale for W_h/W_t in fp8 mode

NCORES = 8
E = 32768
EPC = E // NCORES          # 4096 pairs per core
HID = 512
POOL = 4096
NOBJ = 151
NH = 54                    # 15 + 11 + 24 + 4 head outputs
NB = 8                     # pair blocks per core
BLK = EPC // NB            # 512 pairs per block
KB = HID // 128            # 4 feature chunks of 128
MB = POOL // 128           # 32 pool chunks

GEO = np.array([1, 2, 3, 4, 5, 6, 8, 10, 22, 23, 29, 31, 32, 33, 43], np.int64)
POS = np.array([9, 16, 17, 20, 27, 30, 36, 42, 48, 49, 50], np.int64)
SEM = np.array([7, 11, 12, 13, 14, 15, 18, 19, 21, 24, 25, 26, 28, 34, 35,
                37, 38, 39, 40, 41, 44, 45, 46, 47], np.int64)

_state: dict = {}


def _build():
    """Build + compile the per-core Bass program (same program on all cores)."""
    import concourse.bacc as bacc
    import concourse.mybir as mybir
    from concourse import bass, tile

    dt = mybir.dt
    xdt = dt.float8e4 if USE_FP8 else dt.bfloat16
    nc = bacc.Bacc("TRN2", target_bir_lowering=False, debug=False)

    if USE_FP8:
        # (side, rpair, part, 2, pool)
        w2_d = nc.dram_tensor("w2", [2, KB // 2, 128, 2, POOL], xdt, kind="ExternalInput").ap()
    else:
        w2_d = nc.dram_tensor("w2", [2, KB, 128, POOL], xdt, kind="ExternalInput").ap()
    wc_d = nc.dram_tensor("wc", [128, MB, NH], dt.bfloat16, kind="ExternalInput").ap()
    wv_d = nc.dram_tensor("wv", [128, MB, NH], dt.bfloat16, kind="ExternalInput").ap()
    bc_d = nc.dram_tensor("bcomb", [128, MB], dt.float32, kind="ExternalInput").ap()
    xh_d = nc.dram_tensor("xh", [NB, 128, KB, BLK], xdt, kind="ExternalInput").ap()
    xt_d = nc.dram_tensor("xt", [NB, 128, KB, BLK], xdt, kind="ExternalInput").ap()
    vis_d = nc.dram_tensor("vis", [NB, 128, MB, BLK], dt.bfloat16, kind="ExternalInput").ap()
    b54_d = nc.dram_tensor("b54", [NB, NH, BLK], dt.float32, kind="ExternalInput").ap()
    out_d = nc.dram_tensor("out", [NB, NH, BLK], dt.float32, kind="ExternalOutput").ap()

    NCHUNK = KB // 2 if USE_FP8 else KB   # stationary chunks per side
    DR = mybir.MatmulPerfMode.DoubleRow if USE_FP8 else None

    pdt = dt.bfloat16
    VSPLIT = 8                 # vis DMA split so vis-head MMs start early

    with tile.TileContext(nc) as tc:
        with (
            tc.tile_pool(name="const", bufs=1) as cpool,
            tc.tile_pool(name="xin", bufs=2) as xpool,
            tc.tile_pool(name="vin", bufs=2) as vpool,
            tc.tile_pool(name="bin", bufs=2) as bpool,
            tc.tile_pool(name="post", bufs=6) as ppool,
            tc.tile_pool(name="outp", bufs=2) as opool,
            tc.tile_pool(name="mm", bufs=4, space=bass.MemorySpace.PSUM) as mmpool,
            tc.tile_pool(name="hp", bufs=3, space=bass.MemorySpace.PSUM) as hppool,
        ):
            wv_t = cpool.tile([128, MB, NH], dt.bfloat16, tag="wv")
            nc.sync.dma_start(wv_t[:], wv_d[:])

            def mk_block_tiles(n):
                vis_t = vpool.tile([128, MB, BLK], dt.bfloat16, tag="vis")
                step = MB // VSPLIT
                for v in range(VSPLIT):
                    nc.sync.dma_start(vis_t[:, v * step:(v + 1) * step, :],
                                      vis_d[n, :, v * step:(v + 1) * step, :])
                xh_t = xpool.tile([128, KB, BLK], xdt, tag="xh")
                nc.sync.dma_start(xh_t[:], xh_d[n])
                xt_t = xpool.tile([128, KB, BLK], xdt, tag="xt")
                nc.sync.dma_start(xt_t[:], xt_d[n])
                b54_t = bpool.tile([NH, BLK], dt.float32, tag="b54")
                nc.sync.dma_start(b54_t[:], b54_d[n])
                hp = hppool.tile([NH, BLK], dt.float32, tag="hp")
                return (n, xh_t, xt_t, vis_t, b54_t, hp)

            # first pair's inputs go into the DMA queues before the big W
            # tiles so the vis heads can start right away
            pending = [mk_block_tiles(0), mk_block_tiles(1)]

            wc_t = cpool.tile([128, MB, NH], dt.bfloat16, tag="wc")
            nc.sync.dma_start(wc_t[:], wc_d[:])
            bc_t = cpool.tile([128, MB], dt.float32, tag="bc")
            nc.sync.dma_start(bc_t[:], bc_d[:])
            w_tiles = []
            for s in range(2):
                row = []
                for k in range(NCHUNK):
                    if USE_FP8:
                        t = cpool.tile([128, 2, POOL], xdt, tag=f"w{s}{k}")
                    else:
                        t = cpool.tile([128, POOL], xdt, tag=f"w{s}{k}")
                    nc.sync.dma_start(t[:], w2_d[s, k])
                    row.append(t)
                w_tiles.append(row)

            for g in range(NB // 2):
                blocks = pending
                # interleave the two blocks' vis-head matmuls so each
                # LDWEIGHTS of a wv chunk feeds two matmuls
                for m in range(MB):
                    for b in range(2):
                        hp, vis_t = blocks[b][5], blocks[b][3]
                        nc.tensor.matmul(hp[:], wv_t[:, m, :], vis_t[:, m, :],
                                         start=(m == 0), stop=False)

                posts = [[None] * MB, [None] * MB]
                for m in range(MB):
                    mps = []
                    for b in range(2):
                        xh_t, xt_t = blocks[b][1], blocks[b][2]
                        mp = mmpool.tile([128, BLK], dt.float32, tag="mp")
                        ctr = 0
                        for s, x_t in ((0, xh_t), (1, xt_t)):
                            for k in range(NCHUNK):
                                if USE_FP8:
                                    lhsT = w_tiles[s][k][:, :, m * 128:(m + 1) * 128]
                                    rhs = x_t[:, 2 * k:2 * k + 2, :]
                                else:
                                    lhsT = w_tiles[s][k][:, m * 128:(m + 1) * 128]
                                    rhs = x_t[:, k, :]
                                nc.tensor.matmul(
                                    mp[:], lhsT, rhs,
                                    start=(ctr == 0), stop=(ctr == 2 * NCHUNK - 1),
                                    perf_mode=DR,
                                )
                                ctr += 1
                        mps.append(mp)
                    for b in range(2):
                        post = ppool.tile([128, BLK], pdt, tag="post")
                        nc.scalar.activation(
                            post[:], mps[b][:], mybir.ActivationFunctionType.Relu,
                            bias=bc_t[:, m:m + 1],
                        )
                        posts[b][m] = post
                    # ctx head runs one m behind so PE never waits on ScalarE
                    if m >= 1:
                        for b in range(2):
                            hp = blocks[b][5]
                            nc.tensor.matmul(hp[:], wc_t[:, m - 1, :], posts[b][m - 1][:],
                                             start=False, stop=False)
                # queue next pair's DMAs before the tail ctx matmuls
                if g + 1 < NB // 2:
                    pending = [mk_block_tiles(2 * g + 2), mk_block_tiles(2 * g + 3)]
                for b in range(2):
                    hp = blocks[b][5]
                    nc.tensor.matmul(hp[:], wc_t[:, MB - 1, :], posts[b][MB - 1][:],
                                     start=False, stop=True)
                for b in range(2):
                    n, _, _, _, b54_t, hp = blocks[b]
                    ot = opool.tile([NH, BLK], dt.float32, tag="ot")
                    nc.vector.tensor_add(ot[:], hp[:], b54_t[:])
                    nc.sync.dma_start(out_d[n], ot[:])

    nc.compile()
    return nc


def _prep(inputs):
    """Host-side: fold weights, build the 54-col bias table, gather, block/transpose."""
    ec = np.ascontiguousarray(np.asarray(inputs["edge_ctx"], F32))
    pidx = np.asarray(inputs["pair_idx"])
    vis = np.asarray(inputs["vis_rep"], F32)
    ppred = np.asarray(inputs["pair_pred"])
    f64 = np.float64
    Wpe = np.asarray(inputs["W_post_emb"], f64)
    bpe = np.asarray(inputs["b_post_emb"], f64)
    Wpc = np.asarray(inputs["W_post_cat"], f64)
    bpc = np.asarray(inputs["b_post_cat"], f64)

    W_h = (Wpe[:, :HID] @ Wpc[:HID]).astype(F32)
    W_t = (Wpe[:, HID:] @ Wpc[HID:]).astype(F32)
    b_comb = (bpe[:HID] @ Wpc[:HID] + bpe[HID:] @ Wpc[HID:] + bpc).astype(F32)

    Wc_all = np.concatenate([np.asarray(inputs["Wc1"], F32), np.asarray(inputs["Wc2"], F32),
                             np.asarray(inputs["Wc3"], F32), np.asarray(inputs["Wcs"], F32)], 1)
    Wv_all = np.concatenate([np.asarray(inputs["Wv1"], F32), np.asarray(inputs["Wv2"], F32),
                             np.asarray(inputs["Wv3"], F32), np.asarray(inputs["Wvs"], F32)], 1)
    bh_all = np.concatenate([
        np.asarray(inputs["bc1"], F32) + np.asarray(inputs["bv1"], F32),
        np.asarray(inputs["bc2"], F32) + np.asarray(inputs["bv2"], F32),
        np.asarray(inputs["bc3"], F32) + np.asarray(inputs["bv3"], F32),
        np.asarray(inputs["bcs"], F32) + np.asarray(inputs["bvs"], F32)])

    T = np.asarray(inputs["freq_table"], f64)
    t1, t2, t3 = T[:, GEO], T[:, POS], T[:, SEM]
    lse = lambda t: np.log(np.exp(t).sum(-1))
    tsup = np.stack([T[:, 0], lse(t1), lse(t2), lse(t3)], 1)
    table54 = np.concatenate([t1, t2, t3, tsup], 1).astype(F32) + bh_all

    lin = ppred[:, 0].astype(np.int64) * NOBJ + ppred[:, 1]
    bias54 = table54[lin]                                   # [E, 54]

    if USE_FP8:
        # fp8 DoubleRow layout: w2[s, r, p, i, col] = W_s[(2r+i)*128+p, col] * WSCALE
        w2 = np.stack([
            (W_h * WSCALE).reshape(KB // 2, 2, 128, POOL).transpose(0, 2, 1, 3),
            (W_t * WSCALE).reshape(KB // 2, 2, 128, POOL).transpose(0, 2, 1, 3),
        ]).astype(FP8)
        psum_scale = XSCALE * WSCALE
        wc = (Wc_all / psum_scale).reshape(MB, 128, NH).transpose(1, 0, 2).astype(BF16)
        bcomb = np.ascontiguousarray((b_comb * psum_scale).reshape(MB, 128).T)
        xdt = FP8
        xs = XSCALE
    else:
        w2 = np.stack([W_h.reshape(KB, 128, POOL),
                       W_t.reshape(KB, 128, POOL)]).astype(BF16)
        wc = Wc_all.reshape(MB, 128, NH).transpose(1, 0, 2).astype(BF16)
        bcomb = np.ascontiguousarray(b_comb.reshape(MB, 128).T)
        xdt = BF16
        xs = 1.0
    wv = Wv_all.reshape(MB, 128, NH).transpose(1, 0, 2).astype(BF16)

    Xh = ec[pidx[:, 0]] * xs                                # [E, 512]
    Xt = ec[pidx[:, 1]] * xs
    xh_dev = Xh.reshape(NCORES, NB, BLK, KB, 128).transpose(0, 1, 4, 3, 2).astype(xdt)
    xt_dev = Xt.reshape(NCORES, NB, BLK, KB, 128).transpose(0, 1, 4, 3, 2).astype(xdt)
    vis_dev = vis.reshape(NCORES, NB, BLK, MB, 128).transpose(0, 1, 4, 3, 2).astype(BF16)
    b54_dev = np.ascontiguousarray(
        bias54.reshape(NCORES, NB, BLK, NH).transpose(0, 1, 3, 2))

    in_maps = []
    for c in range(NCORES):
        in_maps.append({
            "w2": w2, "wc": wc, "wv": wv, "bcomb": bcomb,
            "xh": xh_dev[c], "xt": xt_dev[c], "vis": vis_dev[c],
            "b54": b54_dev[c],
        })
    return in_maps


def kernel(**inputs):
    from concourse.bass_utils import run_bass_kernel_spmd

    nc = _state.get("nc")
    if nc is None:
        nc = _build()
        _state["nc"] = nc

    in_maps = _prep(inputs)
    _state["in_maps"] = in_maps

    res = run_bass_kernel_spmd(nc, in_maps, list(range(NCORES)))
    _state["last_results"] = res

    full = np.concatenate(
        [res.results[c]["out"].transpose(0, 2, 1).reshape(EPC, NH)
         for c in range(NCORES)], 0)
    rel1 = np.ascontiguousarray(full[:, 0:15])
    rel2 = np.ascontiguousarray(full[:, 15:26])
    rel3 = np.ascontiguousarray(full[:, 26:50])
    sup = np.ascontiguousarray(full[:, 50:54])
    return (rel1, rel2, rel3, sup)


# revision 16
# speedup vs baseline: 1.2695x; 1.0577x over previous
"""Trainium2 Bass kernel for CausalAnalysisHierPredictor (scene-graph hier head).

Strategy
--------
Data-parallel over the pair dimension E=32768 across 8 NeuronCores
(4096 pairs/core). All gathers, transposes, and algebraic weight folding
happen on the host; the device runs only dense GEMMs + ReLU + adds.

Algebra (exact restructuring of the reference):
  post_ctx  = relu(ctx_rep @ W_post_cat + b_post_cat)
            = relu(ec[hidx] @ W_h + ec[tidx] @ W_t + b_comb)
      with W_h = Wpe[:, :512] @ Wpc[:512],  W_t = Wpe[:, 512:] @ Wpc[512:]
           b_comb = bpe[:512] @ Wpc[:512] + bpe[512:] @ Wpc[512:] + bpc
  out54     = post_ctx @ [Wc1|Wc2|Wc3|Wcs] + vis @ [Wv1|Wv2|Wv3|Wvs]
              + table54[sub*151 + obj]
      where table54 folds the GEO/POS/SEM column permutation, the
      log-sum-exp "sup" columns and all head biases into one 22801x54
      lookup table (pure per-row function of freq_table).

Device layout per core (pairs on the matmul free axis):
  post_ctx[pool, pair] accumulated in PSUM from stationary W chunks
  x moving X^T chunks [feat, pair512]; ReLU+bias via ScalarE into bf16
  SBUF; both heads accumulate into one PSUM tile [54, 512] per block
  with stationary Wc/Wv chunks [pool128, 54].

  Main GEMM runs in fp8e4m3 with DoubleRow (K=256/matmul); inputs are
  pre-scaled on host (X*16, W*64) to sit in e4m3's normal range, and the
  1024x PSUM scale is folded into the ReLU bias (relu is positively
  homogeneous) and into Wc (divided by 1024). Heads stay bf16.
  Pair blocks are processed two at a time so each LDWEIGHTS feeds two
  matmuls; each block's vis-head matmuls run before the mains so the
  first block can start before the big W tiles finish loading.
"""

import sys

if "/opt/trn_rl_repo" not in sys.path:
    sys.path.insert(0, "/opt/trn_rl_repo")

import numpy as np
import ml_dtypes

BF16 = ml_dtypes.bfloat16
FP8 = ml_dtypes.float8_e4m3
F32 = np.float32

USE_FP8 = True             # fp8e4m3 DoubleRow main GEMM (heads stay bf16)
XSCALE = 16.0              # host pre-scale for X in fp8 mode
WSCALE = 64.0              # host pre-scale for W_h/W_t in fp8 mode

NCORES = 8
E = 32768
EPC = E // NCORES          # 4096 pairs per core
HID = 512
POOL = 4096
NOBJ = 151
NH = 54                    # 15 + 11 + 24 + 4 head outputs
NB = 8                     # pair blocks per core
BLK = EPC // NB            # 512 pairs per block
KB = HID // 128            # 4 feature chunks of 128
MB = POOL // 128           # 32 pool chunks

GEO = np.array([1, 2, 3, 4, 5, 6, 8, 10, 22, 23, 29, 31, 32, 33, 43], np.int64)
POS = np.array([9, 16, 17, 20, 27, 30, 36, 42, 48, 49, 50], np.int64)
SEM = np.array([7, 11, 12, 13, 14, 15, 18, 19, 21, 24, 25, 26, 28, 34, 35,
                37, 38, 39, 40, 41, 44, 45, 46, 47], np.int64)

_state: dict = {}


def _build():
    """Build + compile the per-core Bass program (same program on all cores)."""
    import concourse.bacc as bacc
    import concourse.mybir as mybir
    from concourse import bass, tile

    dt = mybir.dt
    xdt = dt.float8e4 if USE_FP8 else dt.bfloat16
    nc = bacc.Bacc("TRN2", target_bir_lowering=False, debug=False)

    if USE_FP8:
        # (side, rpair, part, 2, pool)
        w2_d = nc.dram_tensor("w2", [2, KB // 2, 128, 2, POOL], xdt, kind="ExternalInput").ap()
    else:
        w2_d = nc.dram_tensor("w2", [2, KB, 128, POOL], xdt, kind="ExternalInput").ap()
    wc_d = nc.dram_tensor("wc", [128, MB, NH], dt.bfloat16, kind="ExternalInput").ap()
    wv_d = nc.dram_tensor("wv", [128, MB, NH], dt.bfloat16, kind="ExternalInput").ap()
    bc_d = nc.dram_tensor("bcomb", [128, MB], dt.float32, kind="ExternalInput").ap()
    xh_d = nc.dram_tensor("xh", [NB, 128, KB, BLK], xdt, kind="ExternalInput").ap()
    xt_d = nc.dram_tensor("xt", [NB, 128, KB, BLK], xdt, kind="ExternalInput").ap()
    vis_d = nc.dram_tensor("vis", [NB, 128, MB, BLK], dt.bfloat16, kind="ExternalInput").ap()
    b54_d = nc.dram_tensor("b54", [NB, NH, BLK], dt.float32, kind="ExternalInput").ap()
    out_d = nc.dram_tensor("out", [NB, NH, BLK], dt.float32, kind="ExternalOutput").ap()

    NCHUNK = KB // 2 if USE_FP8 else KB   # stationary chunks per side
    DR = mybir.MatmulPerfMode.DoubleRow if USE_FP8 else None

    pdt = dt.bfloat16
    VSPLIT = 4                 # vis DMA split so vis-head MMs start early

    with tile.TileContext(nc) as tc:
        with (
            tc.tile_pool(name="const", bufs=1) as cpool,
            tc.tile_pool(name="xin", bufs=2) as xpool,
            tc.tile_pool(name="vin", bufs=2) as vpool,
            tc.tile_pool(name="bin", bufs=2) as bpool,
            tc.tile_pool(name="post", bufs=6) as ppool,
            tc.tile_pool(name="outp", bufs=2) as opool,
            tc.tile_pool(name="mm", bufs=4, space=bass.MemorySpace.PSUM) as mmpool,
            tc.tile_pool(name="hp", bufs=3, space=bass.MemorySpace.PSUM) as hppool,
        ):
            wv_t = cpool.tile([128, MB, NH], dt.bfloat16, tag="wv")
            nc.sync.dma_start(wv_t[:], wv_d[:])

            def mk_pair_tiles(n0):
                # both blocks' vis DMAs first: the vis heads consume them
                # before anything else needs xh/xt
                vts = []
                for n in (n0, n0 + 1):
                    vis_t = vpool.tile([128, MB, BLK], dt.bfloat16, tag="vis")
                    step = MB // VSPLIT
                    for v in range(VSPLIT):
                        nc.sync.dma_start(vis_t[:, v * step:(v + 1) * step, :],
                                          vis_d[n, :, v * step:(v + 1) * step, :])
                    vts.append(vis_t)
                out = []
                for b, n in ((0, n0), (1, n0 + 1)):
                    xh_t = xpool.tile([128, KB, BLK], xdt, tag="xh")
                    nc.sync.dma_start(xh_t[:], xh_d[n])
                    xt_t = xpool.tile([128, KB, BLK], xdt, tag="xt")
                    nc.sync.dma_start(xt_t[:], xt_d[n])
                    b54_t = bpool.tile([NH, BLK], dt.float32, tag="b54")
                    nc.sync.dma_start(b54_t[:], b54_d[n])
                    hp = hppool.tile([NH, BLK], dt.float32, tag="hp")
                    out.append((n, xh_t, xt_t, vts[b], b54_t, hp))
                return out

            # first pair's inputs go into the DMA queues before the big W
            # tiles so the vis heads can start right away
            pending = mk_pair_tiles(0)

            wc_t = cpool.tile([128, MB, NH], dt.bfloat16, tag="wc")
            nc.sync.dma_start(wc_t[:], wc_d[:])
            bc_t = cpool.tile([128, MB], dt.float32, tag="bc")
            nc.sync.dma_start(bc_t[:], bc_d[:])
            w_tiles = []
            for s in range(2):
                row = []
                for k in range(NCHUNK):
                    if USE_FP8:
                        t = cpool.tile([128, 2, POOL], xdt, tag=f"w{s}{k}")
                    else:
                        t = cpool.tile([128, POOL], xdt, tag=f"w{s}{k}")
                    nc.sync.dma_start(t[:], w2_d[s, k])
                    row.append(t)
                w_tiles.append(row)

            for g in range(NB // 2):
                blocks = pending
                # interleave the two blocks' vis-head matmuls so each
                # LDWEIGHTS of a wv chunk feeds two matmuls
                for m in range(MB):
                    for b in range(2):
                        hp, vis_t = blocks[b][5], blocks[b][3]
                        nc.tensor.matmul(hp[:], wv_t[:, m, :], vis_t[:, m, :],
                                         start=(m == 0), stop=False)

                posts = [[None] * MB, [None] * MB]
                for m in range(MB):
                    mps = []
                    for b in range(2):
                        xh_t, xt_t = blocks[b][1], blocks[b][2]
                        mp = mmpool.tile([128, BLK], dt.float32, tag="mp")
                        ctr = 0
                        for s, x_t in ((0, xh_t), (1, xt_t)):
                            for k in range(NCHUNK):
                                if USE_FP8:
                                    lhsT = w_tiles[s][k][:, :, m * 128:(m + 1) * 128]
                                    rhs = x_t[:, 2 * k:2 * k + 2, :]
                                else:
                                    lhsT = w_tiles[s][k][:, m * 128:(m + 1) * 128]
                                    rhs = x_t[:, k, :]
                                nc.tensor.matmul(
                                    mp[:], lhsT, rhs,
                                    start=(ctr == 0), stop=(ctr == 2 * NCHUNK - 1),
                                    perf_mode=DR,
                                )
                                ctr += 1
                        mps.append(mp)
                    # relu+bias split across ScalarE (block a) and
                    # VectorE (block b) so neither engine gates the ctx MMs
                    post = ppool.tile([128, BLK], pdt, tag="post")
                    nc.scalar.activation(
                        post[:], mps[0][:], mybir.ActivationFunctionType.Relu,
                        bias=bc_t[:, m:m + 1],
                    )
                    posts[0][m] = post
                    post = ppool.tile([128, BLK], pdt, tag="post")
                    nc.vector.tensor_scalar(
                        post[:], mps[1][:], bc_t[:, m:m + 1], 0.0,
                        mybir.AluOpType.add, mybir.AluOpType.max,
                    )
                    posts[1][m] = post
                    # ctx head: batched per two m, lag >= 2, newest tick
                    # first so Tile emits one wait per engine per batch
                    if m >= 3 and m % 2 == 1:
                        for b in range(2):
                            hp = blocks[b][5]
                            for pm in (m - 2, m - 3):
                                nc.tensor.matmul(hp[:], wc_t[:, pm, :], posts[b][pm][:],
                                                 start=False, stop=False)
                # queue next pair's DMAs before the tail ctx matmuls
                if g + 1 < NB // 2:
                    pending = mk_pair_tiles(2 * g + 2)
                for b in range(2):
                    hp = blocks[b][5]
                    for pm in (MB - 2, MB - 1):
                        nc.tensor.matmul(hp[:], wc_t[:, pm, :], posts[b][pm][:],
                                         start=False, stop=(pm == MB - 1))
                for b in range(2):
                    n, _, _, _, b54_t, hp = blocks[b]
                    ot = opool.tile([NH, BLK], dt.float32, tag="ot")
                    nc.vector.tensor_add(ot[:], hp[:], b54_t[:])
                    nc.sync.dma_start(out_d[n], ot[:])

    nc.compile()
    return nc


def _prep(inputs):
    """Host-side: fold weights, build the 54-col bias table, gather, block/transpose."""
    ec = np.ascontiguousarray(np.asarray(inputs["edge_ctx"], F32))
    pidx = np.asarray(inputs["pair_idx"])
    vis = np.asarray(inputs["vis_rep"], F32)
    ppred = np.asarray(inputs["pair_pred"])
    f64 = np.float64
    Wpe = np.asarray(inputs["W_post_emb"], f64)
    bpe = np.asarray(inputs["b_post_emb"], f64)
    Wpc = np.asarray(inputs["W_post_cat"], f64)
    bpc = np.asarray(inputs["b_post_cat"], f64)

    W_h = (Wpe[:, :HID] @ Wpc[:HID]).astype(F32)
    W_t = (Wpe[:, HID:] @ Wpc[HID:]).astype(F32)
    b_comb = (bpe[:HID] @ Wpc[:HID] + bpe[HID:] @ Wpc[HID:] + bpc).astype(F32)

    Wc_all = np.concatenate([np.asarray(inputs["Wc1"], F32), np.asarray(inputs["Wc2"], F32),
                             np.asarray(inputs["Wc3"], F32), np.asarray(inputs["Wcs"], F32)], 1)
    Wv_all = np.concatenate([np.asarray(inputs["Wv1"], F32), np.asarray(inputs["Wv2"], F32),
                             np.asarray(inputs["Wv3"], F32), np.asarray(inputs["Wvs"], F32)], 1)
    bh_all = np.concatenate([
        np.asarray(inputs["bc1"], F32) + np.asarray(inputs["bv1"], F32),
        np.asarray(inputs["bc2"], F32) + np.asarray(inputs["bv2"], F32),
        np.asarray(inputs["bc3"], F32) + np.asarray(inputs["bv3"], F32),
        np.asarray(inputs["bcs"], F32) + np.asarray(inputs["bvs"], F32)])

    T = np.asarray(inputs["freq_table"], f64)
    t1, t2, t3 = T[:, GEO], T[:, POS], T[:, SEM]
    lse = lambda t: np.log(np.exp(t).sum(-1))
    tsup = np.stack([T[:, 0], lse(t1), lse(t2), lse(t3)], 1)
    table54 = np.concatenate([t1, t2, t3, tsup], 1).astype(F32) + bh_all

    lin = ppred[:, 0].astype(np.int64) * NOBJ + ppred[:, 1]
    bias54 = table54[lin]                                   # [E, 54]

    if USE_FP8:
        # fp8 DoubleRow layout: w2[s, r, p, i, col] = W_s[(2r+i)*128+p, col] * WSCALE
        w2 = np.stack([
            (W_h * WSCALE).reshape(KB // 2, 2, 128, POOL).transpose(0, 2, 1, 3),
            (W_t * WSCALE).reshape(KB // 2, 2, 128, POOL).transpose(0, 2, 1, 3),
        ]).astype(FP8)
        psum_scale = XSCALE * WSCALE
        wc = (Wc_all / psum_scale).reshape(MB, 128, NH).transpose(1, 0, 2).astype(BF16)
        bcomb = np.ascontiguousarray((b_comb * psum_scale).reshape(MB, 128).T)
        xdt = FP8
        xs = XSCALE
    else:
        w2 = np.stack([W_h.reshape(KB, 128, POOL),
                       W_t.reshape(KB, 128, POOL)]).astype(BF16)
        wc = Wc_all.reshape(MB, 128, NH).transpose(1, 0, 2).astype(BF16)
        bcomb = np.ascontiguousarray(b_comb.reshape(MB, 128).T)
        xdt = BF16
        xs = 1.0
    wv = Wv_all.reshape(MB, 128, NH).transpose(1, 0, 2).astype(BF16)

    Xh = ec[pidx[:, 0]] * xs                                # [E, 512]
    Xt = ec[pidx[:, 1]] * xs
    xh_dev = Xh.reshape(NCORES, NB, BLK, KB, 128).transpose(0, 1, 4, 3, 2).astype(xdt)
    xt_dev = Xt.reshape(NCORES, NB, BLK, KB, 128).transpose(0, 1, 4, 3, 2).astype(xdt)
    vis_dev = vis.reshape(NCORES, NB, BLK, MB, 128).transpose(0, 1, 4, 3, 2).astype(BF16)
    b54_dev = np.ascontiguousarray(
        bias54.reshape(NCORES, NB, BLK, NH).transpose(0, 1, 3, 2))

    in_maps = []
    for c in range(NCORES):
        in_maps.append({
            "w2": w2, "wc": wc, "wv": wv, "bcomb": bcomb,
            "xh": xh_dev[c], "xt": xt_dev[c], "vis": vis_dev[c],
            "b54": b54_dev[c],
        })
    return in_maps


def kernel(**inputs):
    from concourse.bass_utils import run_bass_kernel_spmd

    nc = _state.get("nc")
    if nc is None:
        nc = _build()
        _state["nc"] = nc

    in_maps = _prep(inputs)
    _state["in_maps"] = in_maps

    res = run_bass_kernel_spmd(nc, in_maps, list(range(NCORES)))
    _state["last_results"] = res

    full = np.concatenate(
        [res.results[c]["out"].transpose(0, 2, 1).reshape(EPC, NH)
         for c in range(NCORES)], 0)
    rel1 = np.ascontiguousarray(full[:, 0:15])
    rel2 = np.ascontiguousarray(full[:, 15:26])
    rel3 = np.ascontiguousarray(full[:, 26:50])
    sup = np.ascontiguousarray(full[:, 50:54])
    return (rel1, rel2, rel3, sup)


# revision 18
# speedup vs baseline: 1.2785x; 1.0071x over previous
"""Trainium2 Bass kernel for CausalAnalysisHierPredictor (scene-graph hier head).

Strategy
--------
Data-parallel over the pair dimension E=32768 across 8 NeuronCores
(4096 pairs/core). All gathers, transposes, and algebraic weight folding
happen on the host; the device runs only dense GEMMs + ReLU + adds.

Algebra (exact restructuring of the reference):
  post_ctx  = relu(ctx_rep @ W_post_cat + b_post_cat)
            = relu(ec[hidx] @ W_h + ec[tidx] @ W_t + b_comb)
      with W_h = Wpe[:, :512] @ Wpc[:512],  W_t = Wpe[:, 512:] @ Wpc[512:]
           b_comb = bpe[:512] @ Wpc[:512] + bpe[512:] @ Wpc[512:] + bpc
  out54     = post_ctx @ [Wc1|Wc2|Wc3|Wcs] + vis @ [Wv1|Wv2|Wv3|Wvs]
              + table54[sub*151 + obj]
      where table54 folds the GEO/POS/SEM column permutation, the
      log-sum-exp "sup" columns and all head biases into one 22801x54
      lookup table (pure per-row function of freq_table).

Device layout per core (pairs on the matmul free axis):
  post_ctx[pool, pair] accumulated in PSUM from stationary W chunks
  x moving X^T chunks [feat, pair512]; ReLU+bias via ScalarE into bf16
  SBUF; both heads accumulate into one PSUM tile [54, 512] per block
  with stationary Wc/Wv chunks [pool128, 54].

  Main GEMM runs in fp8e4m3 with DoubleRow (K=256/matmul); inputs are
  pre-scaled on host (X*16, W*64) to sit in e4m3's normal range, and the
  1024x PSUM scale is folded into the ReLU bias (relu is positively
  homogeneous) and into Wc (divided by 1024). Heads stay bf16.
  Pair blocks are processed two at a time so each LDWEIGHTS feeds two
  matmuls; each block's vis-head matmuls run before the mains so the
  first block can start before the big W tiles finish loading.
"""

import sys

if "/opt/trn_rl_repo" not in sys.path:
    sys.path.insert(0, "/opt/trn_rl_repo")

import numpy as np
import ml_dtypes

BF16 = ml_dtypes.bfloat16
FP8 = ml_dtypes.float8_e4m3
F32 = np.float32

USE_FP8 = True             # fp8e4m3 DoubleRow main GEMM (heads stay bf16)
XSCALE = 16.0              # host pre-scale for X in fp8 mode
WSCALE = 64.0              # host pre-scale for W_h/W_t in fp8 mode

NCORES = 8
E = 32768
EPC = E // NCORES          # 4096 pairs per core
HID = 512
POOL = 4096
NOBJ = 151
NH = 54                    # 15 + 11 + 24 + 4 head outputs
NB = 8                     # pair blocks per core
BLK = EPC // NB            # 512 pairs per block
KB = HID // 128            # 4 feature chunks of 128
MB = POOL // 128           # 32 pool chunks

GEO = np.array([1, 2, 3, 4, 5, 6, 8, 10, 22, 23, 29, 31, 32, 33, 43], np.int64)
POS = np.array([9, 16, 17, 20, 27, 30, 36, 42, 48, 49, 50], np.int64)
SEM = np.array([7, 11, 12, 13, 14, 15, 18, 19, 21, 24, 25, 26, 28, 34, 35,
                37, 38, 39, 40, 41, 44, 45, 46, 47], np.int64)

_state: dict = {}


def _build():
    """Build + compile the per-core Bass program (same program on all cores)."""
    import concourse.bacc as bacc
    import concourse.mybir as mybir
    from concourse import bass, tile

    dt = mybir.dt
    xdt = dt.float8e4 if USE_FP8 else dt.bfloat16
    nc = bacc.Bacc("TRN2", target_bir_lowering=False, debug=False)

    if USE_FP8:
        # (side, rpair, part, 2, pool)
        w2_d = nc.dram_tensor("w2", [2, KB // 2, 128, 2, POOL], xdt, kind="ExternalInput").ap()
    else:
        w2_d = nc.dram_tensor("w2", [2, KB, 128, POOL], xdt, kind="ExternalInput").ap()
    wc_d = nc.dram_tensor("wc", [128, MB, NH], dt.bfloat16, kind="ExternalInput").ap()
    wv_d = nc.dram_tensor("wv", [128, MB, NH], dt.bfloat16, kind="ExternalInput").ap()
    bc_d = nc.dram_tensor("bcomb", [128, MB], dt.float32, kind="ExternalInput").ap()
    xh_d = nc.dram_tensor("xh", [NB, 128, KB, BLK], xdt, kind="ExternalInput").ap()
    xt_d = nc.dram_tensor("xt", [NB, 128, KB, BLK], xdt, kind="ExternalInput").ap()
    vis_d = nc.dram_tensor("vis", [NB, 128, MB, BLK], dt.bfloat16, kind="ExternalInput").ap()
    b54_d = nc.dram_tensor("b54", [NB, NH, BLK], dt.float32, kind="ExternalInput").ap()
    out_d = nc.dram_tensor("out", [NB, NH, BLK], dt.float32, kind="ExternalOutput").ap()

    NCHUNK = KB // 2 if USE_FP8 else KB   # stationary chunks per side
    DR = mybir.MatmulPerfMode.DoubleRow if USE_FP8 else None

    pdt = dt.bfloat16
    VSPLIT = 4                 # vis DMA split so vis-head MMs start early

    with tile.TileContext(nc) as tc:
        with (
            tc.tile_pool(name="const", bufs=1) as cpool,
            tc.tile_pool(name="xin", bufs=2) as xpool,
            tc.tile_pool(name="vin", bufs=2) as vpool,
            tc.tile_pool(name="bin", bufs=2) as bpool,
            tc.tile_pool(name="post", bufs=14) as ppool,
            tc.tile_pool(name="outp", bufs=2) as opool,
            tc.tile_pool(name="mm", bufs=4, space=bass.MemorySpace.PSUM) as mmpool,
            tc.tile_pool(name="hp", bufs=3, space=bass.MemorySpace.PSUM) as hppool,
        ):
            wv_t = cpool.tile([128, MB, NH], dt.bfloat16, tag="wv")
            nc.sync.dma_start(wv_t[:], wv_d[:])

            def vis_dma(vis_t, n):
                step = MB // VSPLIT
                for v in range(VSPLIT):
                    nc.sync.dma_start(vis_t[:, v * step:(v + 1) * step, :],
                                      vis_d[n, :, v * step:(v + 1) * step, :])

            def mk_pair_tiles(n0, defer_vis_b=False):
                # both blocks' vis DMAs first: the vis heads consume them
                # before anything else needs xh/xt
                vts = []
                for b, n in ((0, n0), (1, n0 + 1)):
                    vis_t = vpool.tile([128, MB, BLK], dt.bfloat16, tag="vis")
                    if not (defer_vis_b and b == 1):
                        vis_dma(vis_t, n)
                    vts.append(vis_t)
                out = []
                for b, n in ((0, n0), (1, n0 + 1)):
                    xh_t = xpool.tile([128, KB, BLK], xdt, tag="xh")
                    nc.sync.dma_start(xh_t[:], xh_d[n])
                    xt_t = xpool.tile([128, KB, BLK], xdt, tag="xt")
                    nc.sync.dma_start(xt_t[:], xt_d[n])
                    b54_t = bpool.tile([NH, BLK], dt.float32, tag="b54")
                    nc.sync.dma_start(b54_t[:], b54_d[n])
                    hp = hppool.tile([NH, BLK], dt.float32, tag="hp")
                    out.append((n, xh_t, xt_t, vts[b], b54_t, hp))
                return out

            # first pair: only block a's vis goes in front of the W tiles;
            # block b's vis loads after them and its vis-head matmuls run
            # after the mains, so the PE is never DMA-starved at startup
            pending = mk_pair_tiles(0, defer_vis_b=True)

            wc_t = cpool.tile([128, MB, NH], dt.bfloat16, tag="wc")
            nc.sync.dma_start(wc_t[:], wc_d[:])
            bc_t = cpool.tile([128, MB], dt.float32, tag="bc")
            nc.sync.dma_start(bc_t[:], bc_d[:])
            w_tiles = []
            for s in range(2):
                row = []
                for k in range(NCHUNK):
                    if USE_FP8:
                        t = cpool.tile([128, 2, POOL], xdt, tag=f"w{s}{k}")
                    else:
                        t = cpool.tile([128, POOL], xdt, tag=f"w{s}{k}")
                    nc.sync.dma_start(t[:], w2_d[s, k])
                    row.append(t)
                w_tiles.append(row)
            vis_dma(pending[1][3], 1)

            for g in range(NB // 2):
                blocks = pending
                first = (g == 0)
                # interleave the two blocks' vis-head matmuls so each
                # LDWEIGHTS of a wv chunk feeds two matmuls (pair 0: block
                # b's vis head is deferred until after the mains)
                for m in range(MB):
                    for b in ((0,) if first else (0, 1)):
                        hp, vis_t = blocks[b][5], blocks[b][3]
                        nc.tensor.matmul(hp[:], wv_t[:, m, :], vis_t[:, m, :],
                                         start=(m == 0), stop=False)
                hp_b_started = not first

                posts = [[None] * MB, [None] * MB]
                for m in range(MB):
                    mps = []
                    for b in range(2):
                        xh_t, xt_t = blocks[b][1], blocks[b][2]
                        mp = mmpool.tile([128, BLK], dt.float32, tag="mp")
                        ctr = 0
                        for s, x_t in ((0, xh_t), (1, xt_t)):
                            for k in range(NCHUNK):
                                if USE_FP8:
                                    lhsT = w_tiles[s][k][:, :, m * 128:(m + 1) * 128]
                                    rhs = x_t[:, 2 * k:2 * k + 2, :]
                                else:
                                    lhsT = w_tiles[s][k][:, m * 128:(m + 1) * 128]
                                    rhs = x_t[:, k, :]
                                nc.tensor.matmul(
                                    mp[:], lhsT, rhs,
                                    start=(ctr == 0), stop=(ctr == 2 * NCHUNK - 1),
                                    perf_mode=DR,
                                )
                                ctr += 1
                        mps.append(mp)
                    # relu+bias split across ScalarE (block a) and
                    # VectorE (block b) so neither engine gates the ctx MMs
                    post = ppool.tile([128, BLK], pdt, tag="post")
                    nc.scalar.activation(
                        post[:], mps[0][:], mybir.ActivationFunctionType.Relu,
                        bias=bc_t[:, m:m + 1],
                    )
                    posts[0][m] = post
                    post = ppool.tile([128, BLK], pdt, tag="post")
                    nc.vector.tensor_scalar(
                        post[:], mps[1][:], bc_t[:, m:m + 1], 0.0,
                        mybir.AluOpType.add, mybir.AluOpType.max,
                    )
                    posts[1][m] = post
                    # ctx head: batched per four m, lag >= 2, newest tick
                    # first so Tile emits one wait per engine per batch
                    if m >= 5 and (m - 1) % 4 == 0:
                        for b in range(2):
                            hp = blocks[b][5]
                            for pm in range(m - 2, m - 6, -1):
                                st = first and b == 1 and not hp_b_started
                                hp_b_started |= (b == 1)
                                nc.tensor.matmul(hp[:], wc_t[:, pm, :], posts[b][pm][:],
                                                 start=st, stop=False)
                # queue next pair's DMAs before the tail ctx matmuls
                if g + 1 < NB // 2:
                    pending = mk_pair_tiles(2 * g + 2)
                for b in range(2):
                    hp = blocks[b][5]
                    for pm in range(MB - 1, MB - 5, -1):
                        # newest tick first; the group-closing stop flag must
                        # ride the last-emitted matmul of the group
                        stop = (pm == MB - 4) and not (first and b == 1)
                        nc.tensor.matmul(hp[:], wc_t[:, pm, :], posts[b][pm][:],
                                         start=False, stop=stop)
                if first:
                    # pair 0, block b: vis head runs now, fully hidden
                    # behind the mains' DMA shadow
                    hp, vis_t = blocks[1][5], blocks[1][3]
                    for m in range(MB):
                        nc.tensor.matmul(hp[:], wv_t[:, m, :], vis_t[:, m, :],
                                         start=False, stop=(m == MB - 1))
                for b in range(2):
                    n, _, _, _, b54_t, hp = blocks[b]
                    ot = opool.tile([NH, BLK], dt.float32, tag="ot")
                    nc.vector.tensor_add(ot[:], hp[:], b54_t[:])
                    nc.sync.dma_start(out_d[n], ot[:])

    nc.compile()
    return nc


def _prep(inputs):
    """Host-side: fold weights, build the 54-col bias table, gather, block/transpose."""
    ec = np.ascontiguousarray(np.asarray(inputs["edge_ctx"], F32))
    pidx = np.asarray(inputs["pair_idx"])
    vis = np.asarray(inputs["vis_rep"], F32)
    ppred = np.asarray(inputs["pair_pred"])
    f64 = np.float64
    Wpe = np.asarray(inputs["W_post_emb"], f64)
    bpe = np.asarray(inputs["b_post_emb"], f64)
    Wpc = np.asarray(inputs["W_post_cat"], f64)
    bpc = np.asarray(inputs["b_post_cat"], f64)

    W_h = (Wpe[:, :HID] @ Wpc[:HID]).astype(F32)
    W_t = (Wpe[:, HID:] @ Wpc[HID:]).astype(F32)
    b_comb = (bpe[:HID] @ Wpc[:HID] + bpe[HID:] @ Wpc[HID:] + bpc).astype(F32)

    Wc_all = np.concatenate([np.asarray(inputs["Wc1"], F32), np.asarray(inputs["Wc2"], F32),
                             np.asarray(inputs["Wc3"], F32), np.asarray(inputs["Wcs"], F32)], 1)
    Wv_all = np.concatenate([np.asarray(inputs["Wv1"], F32), np.asarray(inputs["Wv2"], F32),
                             np.asarray(inputs["Wv3"], F32), np.asarray(inputs["Wvs"], F32)], 1)
    bh_all = np.concatenate([
        np.asarray(inputs["bc1"], F32) + np.asarray(inputs["bv1"], F32),
        np.asarray(inputs["bc2"], F32) + np.asarray(inputs["bv2"], F32),
        np.asarray(inputs["bc3"], F32) + np.asarray(inputs["bv3"], F32),
        np.asarray(inputs["bcs"], F32) + np.asarray(inputs["bvs"], F32)])

    T = np.asarray(inputs["freq_table"], f64)
    t1, t2, t3 = T[:, GEO], T[:, POS], T[:, SEM]
    lse = lambda t: np.log(np.exp(t).sum(-1))
    tsup = np.stack([T[:, 0], lse(t1), lse(t2), lse(t3)], 1)
    table54 = np.concatenate([t1, t2, t3, tsup], 1).astype(F32) + bh_all

    lin = ppred[:, 0].astype(np.int64) * NOBJ + ppred[:, 1]
    bias54 = table54[lin]                                   # [E, 54]

    if USE_FP8:
        # fp8 DoubleRow layout: w2[s, r, p, i, col] = W_s[(2r+i)*128+p, col] * WSCALE
        w2 = np.stack([
            (W_h * WSCALE).reshape(KB // 2, 2, 128, POOL).transpose(0, 2, 1, 3),
            (W_t * WSCALE).reshape(KB // 2, 2, 128, POOL).transpose(0, 2, 1, 3),
        ]).astype(FP8)
        psum_scale = XSCALE * WSCALE
        wc = (Wc_all / psum_scale).reshape(MB, 128, NH).transpose(1, 0, 2).astype(BF16)
        bcomb = np.ascontiguousarray((b_comb * psum_scale).reshape(MB, 128).T)
        xdt = FP8
        xs = XSCALE
    else:
        w2 = np.stack([W_h.reshape(KB, 128, POOL),
                       W_t.reshape(KB, 128, POOL)]).astype(BF16)
        wc = Wc_all.reshape(MB, 128, NH).transpose(1, 0, 2).astype(BF16)
        bcomb = np.ascontiguousarray(b_comb.reshape(MB, 128).T)
        xdt = BF16
        xs = 1.0
    wv = Wv_all.reshape(MB, 128, NH).transpose(1, 0, 2).astype(BF16)

    Xh = ec[pidx[:, 0]] * xs                                # [E, 512]
    Xt = ec[pidx[:, 1]] * xs
    xh_dev = Xh.reshape(NCORES, NB, BLK, KB, 128).transpose(0, 1, 4, 3, 2).astype(xdt)
    xt_dev = Xt.reshape(NCORES, NB, BLK, KB, 128).transpose(0, 1, 4, 3, 2).astype(xdt)
    vis_dev = vis.reshape(NCORES, NB, BLK, MB, 128).transpose(0, 1, 4, 3, 2).astype(BF16)
    b54_dev = np.ascontiguousarray(
        bias54.reshape(NCORES, NB, BLK, NH).transpose(0, 1, 3, 2))

    in_maps = []
    for c in range(NCORES):
        in_maps.append({
            "w2": w2, "wc": wc, "wv": wv, "bcomb": bcomb,
            "xh": xh_dev[c], "xt": xt_dev[c], "vis": vis_dev[c],
            "b54": b54_dev[c],
        })
    return in_maps


def kernel(**inputs):
    from concourse.bass_utils import run_bass_kernel_spmd

    nc = _state.get("nc")
    if nc is None:
        nc = _build()
        _state["nc"] = nc

    in_maps = _prep(inputs)
    _state["in_maps"] = in_maps

    res = run_bass_kernel_spmd(nc, in_maps, list(range(NCORES)))
    _state["last_results"] = res

    full = np.concatenate(
        [res.results[c]["out"].transpose(0, 2, 1).reshape(EPC, NH)
         for c in range(NCORES)], 0)
    rel1 = np.ascontiguousarray(full[:, 0:15])
    rel2 = np.ascontiguousarray(full[:, 15:26])
    rel3 = np.ascontiguousarray(full[:, 26:50])
    sup = np.ascontiguousarray(full[:, 50:54])
    return (rel1, rel2, rel3, sup)


# revision 19
# speedup vs baseline: 1.3165x; 1.0297x over previous
"""Trainium2 Bass kernel for CausalAnalysisHierPredictor (scene-graph hier head).

Strategy
--------
Data-parallel over the pair dimension E=32768 across 8 NeuronCores
(4096 pairs/core). All gathers, transposes, and algebraic weight folding
happen on the host; the device runs only dense GEMMs + ReLU + adds.

Algebra (exact restructuring of the reference):
  post_ctx  = relu(ctx_rep @ W_post_cat + b_post_cat)
            = relu(ec[hidx] @ W_h + ec[tidx] @ W_t + b_comb)
      with W_h = Wpe[:, :512] @ Wpc[:512],  W_t = Wpe[:, 512:] @ Wpc[512:]
           b_comb = bpe[:512] @ Wpc[:512] + bpe[512:] @ Wpc[512:] + bpc
  out54     = post_ctx @ [Wc1|Wc2|Wc3|Wcs] + vis @ [Wv1|Wv2|Wv3|Wvs]
              + table54[sub*151 + obj]
      where table54 folds the GEO/POS/SEM column permutation, the
      log-sum-exp "sup" columns and all head biases into one 22801x54
      lookup table (pure per-row function of freq_table).

Device layout per core (pairs on the matmul free axis):
  post_ctx[pool, pair] accumulated in PSUM from stationary W chunks
  x moving X^T chunks [feat, pair512]; ReLU+bias via ScalarE into bf16
  SBUF; both heads accumulate into one PSUM tile [54, 512] per block
  with stationary Wc/Wv chunks [pool128, 54].

  Main GEMM runs in fp8e4m3 with DoubleRow (K=256/matmul); inputs are
  pre-scaled on host (X*16, W*64) to sit in e4m3's normal range, and the
  1024x PSUM scale is folded into the ReLU bias (relu is positively
  homogeneous) and into Wc (divided by 1024). Heads stay bf16.
  Pair blocks are processed two at a time so each LDWEIGHTS feeds two
  matmuls; each block's vis-head matmuls run before the mains so the
  first block can start before the big W tiles finish loading.
"""

import sys

if "/opt/trn_rl_repo" not in sys.path:
    sys.path.insert(0, "/opt/trn_rl_repo")

import numpy as np
import ml_dtypes

BF16 = ml_dtypes.bfloat16
FP8 = ml_dtypes.float8_e4m3
F32 = np.float32

USE_FP8 = True             # fp8e4m3 DoubleRow main GEMM (heads stay bf16)
XSCALE = 16.0              # host pre-scale for X in fp8 mode
WSCALE = 64.0              # host pre-scale for W_h/W_t in fp8 mode

NCORES = 8
E = 32768
EPC = E // NCORES          # 4096 pairs per core
HID = 512
POOL = 4096
NOBJ = 151
NH = 54                    # 15 + 11 + 24 + 4 head outputs
NB = 8                     # pair blocks per core
BLK = EPC // NB            # 512 pairs per block
KB = HID // 128            # 4 feature chunks of 128
MB = POOL // 128           # 32 pool chunks

GEO = np.array([1, 2, 3, 4, 5, 6, 8, 10, 22, 23, 29, 31, 32, 33, 43], np.int64)
POS = np.array([9, 16, 17, 20, 27, 30, 36, 42, 48, 49, 50], np.int64)
SEM = np.array([7, 11, 12, 13, 14, 15, 18, 19, 21, 24, 25, 26, 28, 34, 35,
                37, 38, 39, 40, 41, 44, 45, 46, 47], np.int64)

_state: dict = {}


def _build():
    """Build + compile the per-core Bass program (same program on all cores)."""
    import concourse.bacc as bacc
    import concourse.mybir as mybir
    from concourse import bass, tile

    dt = mybir.dt
    xdt = dt.float8e4 if USE_FP8 else dt.bfloat16
    nc = bacc.Bacc("TRN2", target_bir_lowering=False, debug=False)

    if USE_FP8:
        # (side, rpair, part, 2, pool)
        w2_d = nc.dram_tensor("w2", [2, KB // 2, 128, 2, POOL], xdt, kind="ExternalInput").ap()
    else:
        w2_d = nc.dram_tensor("w2", [2, KB, 128, POOL], xdt, kind="ExternalInput").ap()
    wc_d = nc.dram_tensor("wc", [128, MB, NH], dt.bfloat16, kind="ExternalInput").ap()
    wv_d = nc.dram_tensor("wv", [128, MB, NH], dt.bfloat16, kind="ExternalInput").ap()
    bc_d = nc.dram_tensor("bcomb", [128, MB], dt.float32, kind="ExternalInput").ap()
    xh_d = nc.dram_tensor("xh", [NB, 128, KB, BLK], xdt, kind="ExternalInput").ap()
    xt_d = nc.dram_tensor("xt", [NB, 128, KB, BLK], xdt, kind="ExternalInput").ap()
    vis_d = nc.dram_tensor("vis", [NB, 128, MB, BLK], dt.bfloat16, kind="ExternalInput").ap()
    b54_d = nc.dram_tensor("b54", [NB, NH, BLK], dt.float32, kind="ExternalInput").ap()
    out_d = nc.dram_tensor("out", [NB, NH, BLK], dt.float32, kind="ExternalOutput").ap()

    NCHUNK = KB // 2 if USE_FP8 else KB   # stationary chunks per side
    DR = mybir.MatmulPerfMode.DoubleRow if USE_FP8 else None

    pdt = dt.bfloat16
    VSPLIT = 4                 # vis DMA split so vis-head MMs start early

    with tile.TileContext(nc) as tc:
        with (
            tc.tile_pool(name="const", bufs=1) as cpool,
            tc.tile_pool(name="xin", bufs=2) as xpool,
            tc.tile_pool(name="vin", bufs=3) as vpool,
            tc.tile_pool(name="bin", bufs=2) as bpool,
            tc.tile_pool(name="post", bufs=14) as ppool,
            tc.tile_pool(name="outp", bufs=2) as opool,
            tc.tile_pool(name="mm", bufs=5, space=bass.MemorySpace.PSUM) as mmpool,
            tc.tile_pool(name="hp", bufs=3, space=bass.MemorySpace.PSUM) as hppool,
        ):
            wv_t = cpool.tile([128, MB, NH], dt.bfloat16, tag="wv")
            nc.sync.dma_start(wv_t[:], wv_d[:])

            def vis_dma(vis_t, n):
                step = MB // VSPLIT
                for v in range(VSPLIT):
                    nc.sync.dma_start(vis_t[:, v * step:(v + 1) * step, :],
                                      vis_d[n, :, v * step:(v + 1) * step, :])

            def mk_pair_tiles(n0, defer_vis_b=False):
                # both blocks' vis DMAs first: the vis heads consume them
                # before anything else needs xh/xt
                vts = []
                for b, n in ((0, n0), (1, n0 + 1)):
                    vis_t = vpool.tile([128, MB, BLK], dt.bfloat16, tag="vis")
                    if not (defer_vis_b and b == 1):
                        vis_dma(vis_t, n)
                    vts.append(vis_t)
                out = []
                for b, n in ((0, n0), (1, n0 + 1)):
                    xh_t = xpool.tile([128, KB, BLK], xdt, tag="xh")
                    nc.sync.dma_start(xh_t[:], xh_d[n])
                    xt_t = xpool.tile([128, KB, BLK], xdt, tag="xt")
                    nc.sync.dma_start(xt_t[:], xt_d[n])
                    b54_t = bpool.tile([NH, BLK], dt.float32, tag="b54")
                    nc.sync.dma_start(b54_t[:], b54_d[n])
                    hp = hppool.tile([NH, BLK], dt.float32, tag="hp")
                    out.append((n, xh_t, xt_t, vts[b], b54_t, hp))
                return out

            # first pair: only block a's vis goes in front of the W tiles;
            # block b's vis loads after them and its vis-head matmuls run
            # after the mains, so the PE is never DMA-starved at startup
            pending = mk_pair_tiles(0, defer_vis_b=True)

            wc_t = cpool.tile([128, MB, NH], dt.bfloat16, tag="wc")
            nc.sync.dma_start(wc_t[:], wc_d[:])
            bc_t = cpool.tile([128, MB], dt.float32, tag="bc")
            nc.sync.dma_start(bc_t[:], bc_d[:])
            w_tiles = []
            for s in range(2):
                row = []
                for k in range(NCHUNK):
                    if USE_FP8:
                        t = cpool.tile([128, 2, POOL], xdt, tag=f"w{s}{k}")
                    else:
                        t = cpool.tile([128, POOL], xdt, tag=f"w{s}{k}")
                    nc.sync.dma_start(t[:], w2_d[s, k])
                    row.append(t)
                w_tiles.append(row)
            vis_dma(pending[1][3], 1)

            for g in range(NB // 2):
                blocks = pending
                first = (g == 0)
                # interleave the two blocks' vis-head matmuls so each
                # LDWEIGHTS of a wv chunk feeds two matmuls (pair 0: block
                # b's vis head is deferred until after the mains)
                for m in range(MB):
                    for b in ((0,) if first else (0, 1)):
                        hp, vis_t = blocks[b][5], blocks[b][3]
                        nc.tensor.matmul(hp[:], wv_t[:, m, :], vis_t[:, m, :],
                                         start=(m == 0), stop=False)
                hp_b_started = not first

                posts = [[None] * MB, [None] * MB]
                for m in range(MB):
                    mps = []
                    for b in range(2):
                        xh_t, xt_t = blocks[b][1], blocks[b][2]
                        mp = mmpool.tile([128, BLK], dt.float32, tag="mp")
                        ctr = 0
                        for s, x_t in ((0, xh_t), (1, xt_t)):
                            for k in range(NCHUNK):
                                if USE_FP8:
                                    lhsT = w_tiles[s][k][:, :, m * 128:(m + 1) * 128]
                                    rhs = x_t[:, 2 * k:2 * k + 2, :]
                                else:
                                    lhsT = w_tiles[s][k][:, m * 128:(m + 1) * 128]
                                    rhs = x_t[:, k, :]
                                nc.tensor.matmul(
                                    mp[:], lhsT, rhs,
                                    start=(ctr == 0), stop=(ctr == 2 * NCHUNK - 1),
                                    perf_mode=DR,
                                )
                                ctr += 1
                        mps.append(mp)
                    # relu+bias split across ScalarE (block a) and
                    # VectorE (block b) so neither engine gates the ctx MMs
                    post = ppool.tile([128, BLK], pdt, tag="post")
                    nc.scalar.activation(
                        post[:], mps[0][:], mybir.ActivationFunctionType.Relu,
                        bias=bc_t[:, m:m + 1],
                    )
                    posts[0][m] = post
                    post = ppool.tile([128, BLK], pdt, tag="post")
                    nc.vector.tensor_scalar(
                        post[:], mps[1][:], bc_t[:, m:m + 1], 0.0,
                        mybir.AluOpType.add, mybir.AluOpType.max,
                    )
                    posts[1][m] = post
                    # ctx head: batched per four m, lag >= 2, newest tick
                    # first so Tile emits one wait per engine per batch
                    if m >= 5 and (m - 1) % 4 == 0:
                        for b in range(2):
                            hp = blocks[b][5]
                            for pm in range(m - 2, m - 6, -1):
                                st = first and b == 1 and not hp_b_started
                                hp_b_started |= (b == 1)
                                nc.tensor.matmul(hp[:], wc_t[:, pm, :], posts[b][pm][:],
                                                 start=st, stop=False)
                # queue next pair's DMAs before the tail ctx matmuls
                if g + 1 < NB // 2:
                    pending = mk_pair_tiles(2 * g + 2)
                for b in range(2):
                    hp = blocks[b][5]
                    for pm in range(MB - 1, MB - 5, -1):
                        # newest tick first; the group-closing stop flag must
                        # ride the last-emitted matmul of the group
                        stop = (pm == MB - 4) and not (first and b == 1)
                        nc.tensor.matmul(hp[:], wc_t[:, pm, :], posts[b][pm][:],
                                         start=False, stop=stop)
                if first:
                    # pair 0, block b: vis head runs now, fully hidden
                    # behind the mains' DMA shadow
                    hp, vis_t = blocks[1][5], blocks[1][3]
                    for m in range(MB):
                        nc.tensor.matmul(hp[:], wv_t[:, m, :], vis_t[:, m, :],
                                         start=False, stop=(m == MB - 1))
                for b in range(2):
                    n, _, _, _, b54_t, hp = blocks[b]
                    ot = opool.tile([NH, BLK], dt.float32, tag="ot")
                    nc.vector.tensor_add(ot[:], hp[:], b54_t[:])
                    nc.sync.dma_start(out_d[n], ot[:])

    nc.compile()
    return nc


def _prep(inputs):
    """Host-side: fold weights, build the 54-col bias table, gather, block/transpose."""
    ec = np.ascontiguousarray(np.asarray(inputs["edge_ctx"], F32))
    pidx = np.asarray(inputs["pair_idx"])
    vis = np.asarray(inputs["vis_rep"], F32)
    ppred = np.asarray(inputs["pair_pred"])
    f64 = np.float64
    Wpe = np.asarray(inputs["W_post_emb"], f64)
    bpe = np.asarray(inputs["b_post_emb"], f64)
    Wpc = np.asarray(inputs["W_post_cat"], f64)
    bpc = np.asarray(inputs["b_post_cat"], f64)

    W_h = (Wpe[:, :HID] @ Wpc[:HID]).astype(F32)
    W_t = (Wpe[:, HID:] @ Wpc[HID:]).astype(F32)
    b_comb = (bpe[:HID] @ Wpc[:HID] + bpe[HID:] @ Wpc[HID:] + bpc).astype(F32)

    Wc_all = np.concatenate([np.asarray(inputs["Wc1"], F32), np.asarray(inputs["Wc2"], F32),
                             np.asarray(inputs["Wc3"], F32), np.asarray(inputs["Wcs"], F32)], 1)
    Wv_all = np.concatenate([np.asarray(inputs["Wv1"], F32), np.asarray(inputs["Wv2"], F32),
                             np.asarray(inputs["Wv3"], F32), np.asarray(inputs["Wvs"], F32)], 1)
    bh_all = np.concatenate([
        np.asarray(inputs["bc1"], F32) + np.asarray(inputs["bv1"], F32),
        np.asarray(inputs["bc2"], F32) + np.asarray(inputs["bv2"], F32),
        np.asarray(inputs["bc3"], F32) + np.asarray(inputs["bv3"], F32),
        np.asarray(inputs["bcs"], F32) + np.asarray(inputs["bvs"], F32)])

    T = np.asarray(inputs["freq_table"], f64)
    t1, t2, t3 = T[:, GEO], T[:, POS], T[:, SEM]
    lse = lambda t: np.log(np.exp(t).sum(-1))
    tsup = np.stack([T[:, 0], lse(t1), lse(t2), lse(t3)], 1)
    table54 = np.concatenate([t1, t2, t3, tsup], 1).astype(F32) + bh_all

    lin = ppred[:, 0].astype(np.int64) * NOBJ + ppred[:, 1]
    bias54 = table54[lin]                                   # [E, 54]

    if USE_FP8:
        # fp8 DoubleRow layout: w2[s, r, p, i, col] = W_s[(2r+i)*128+p, col] * WSCALE
        w2 = np.stack([
            (W_h * WSCALE).reshape(KB // 2, 2, 128, POOL).transpose(0, 2, 1, 3),
            (W_t * WSCALE).reshape(KB // 2, 2, 128, POOL).transpose(0, 2, 1, 3),
        ]).astype(FP8)
        psum_scale = XSCALE * WSCALE
        wc = (Wc_all / psum_scale).reshape(MB, 128, NH).transpose(1, 0, 2).astype(BF16)
        bcomb = np.ascontiguousarray((b_comb * psum_scale).reshape(MB, 128).T)
        xdt = FP8
        xs = XSCALE
    else:
        w2 = np.stack([W_h.reshape(KB, 128, POOL),
                       W_t.reshape(KB, 128, POOL)]).astype(BF16)
        wc = Wc_all.reshape(MB, 128, NH).transpose(1, 0, 2).astype(BF16)
        bcomb = np.ascontiguousarray(b_comb.reshape(MB, 128).T)
        xdt = BF16
        xs = 1.0
    wv = Wv_all.reshape(MB, 128, NH).transpose(1, 0, 2).astype(BF16)

    Xh = ec[pidx[:, 0]] * xs                                # [E, 512]
    Xt = ec[pidx[:, 1]] * xs
    xh_dev = Xh.reshape(NCORES, NB, BLK, KB, 128).transpose(0, 1, 4, 3, 2).astype(xdt)
    xt_dev = Xt.reshape(NCORES, NB, BLK, KB, 128).transpose(0, 1, 4, 3, 2).astype(xdt)
    vis_dev = vis.reshape(NCORES, NB, BLK, MB, 128).transpose(0, 1, 4, 3, 2).astype(BF16)
    b54_dev = np.ascontiguousarray(
        bias54.reshape(NCORES, NB, BLK, NH).transpose(0, 1, 3, 2))

    in_maps = []
    for c in range(NCORES):
        in_maps.append({
            "w2": w2, "wc": wc, "wv": wv, "bcomb": bcomb,
            "xh": xh_dev[c], "xt": xt_dev[c], "vis": vis_dev[c],
            "b54": b54_dev[c],
        })
    return in_maps


def kernel(**inputs):
    from concourse.bass_utils import run_bass_kernel_spmd

    nc = _state.get("nc")
    if nc is None:
        nc = _build()
        _state["nc"] = nc

    in_maps = _prep(inputs)
    _state["in_maps"] = in_maps

    res = run_bass_kernel_spmd(nc, in_maps, list(range(NCORES)))
    _state["last_results"] = res

    full = np.concatenate(
        [res.results[c]["out"].transpose(0, 2, 1).reshape(EPC, NH)
         for c in range(NCORES)], 0)
    rel1 = np.ascontiguousarray(full[:, 0:15])
    rel2 = np.ascontiguousarray(full[:, 15:26])
    rel3 = np.ascontiguousarray(full[:, 26:50])
    sup = np.ascontiguousarray(full[:, 50:54])
    return (rel1, rel2, rel3, sup)
